# revision 36
# baseline (speedup 1.0000x reference)
"""Trainium2 Bass kernel for DiagTrainableLDAHead (retrieval_knn).

out[n,c] = log_prior[c] - 0.5*(m2[n,c] + log_det)
m2[n,c]  = sum_d (z[n,d]-mu[c,d])^2 * inv_var[d]

=> out[n,c] = cross[n,c] + cb[c] + rb[n]
   cross = (z*iv) @ mu^T          fp8 DoubleRow GEMM (2 k-passes of 256)
   cb[c] = prior[c] - 0.5*sum_d mu[c,d]^2 iv[d]
           computed on the PE ([128,2,1] -iv/2 stationary over mu^2 fp8 +
           prior rank-1), then ACCUMULATED INTO EACH PSUM CHUNK by a
           trailing fp32r rank-1 matmul (ones x cb) in the same
           accumulation group -- so eviction needs no second tensor input.
   rb[n] = -0.5*sum_d z[n,d]^2 iv[d] - logsumexp(prior) - 0.5*log_det
           applied as the [P,1] bias of the single-pass eviction.

Eviction: psum [128,1024] (2 banks) -> bf16 SBUF, alternating
scalar.activation(bias=rb) / vector.tensor_scalar_add(rb). gpsimd cannot
read PSUM (hardware restriction) so it handles SBUF-only mu^2 squares.

Sharding: data-parallel over N across 8 cores; mu/log_cov/prior
replicated; no collectives. Host prep is layout/dtype-only; output is
computed in bf16 and widened to fp32 on the host (|out|~350, bf16 ulp~1,
tolerance ~7).
"""
import sys

sys.path.insert(0, "/opt/trn_rl_repo")

import numpy as np
import ml_dtypes

import concourse.bacc as bacc
import concourse.tile as tile
from concourse import mybir
from concourse import bass_isa
from concourse.bass_utils import run_bass_kernel_spmd

F32 = mybir.dt.float32
F32R = mybir.dt.float32r
BF16 = mybir.dt.bfloat16
FP8 = mybir.dt.float8e4
AF = mybir.ActivationFunctionType
ALU = mybir.AluOpType
DR = mybir.MatmulPerfMode.DoubleRow

N, C, D = 8192, 2048, 512
NCORES = 8
NSH = N // NCORES          # 1024 rows per core
P = 128                    # partitions
KT = D // P                # 4 k-tiles
NT = NSH // P              # 8 n-tiles
F = 512                    # single PSUM bank chunk
F2 = 2 * F                 # eviction chunk (2 banks)
CJ = C // F                # 4 c-chunks
CP = C // F2               # 2 c-pair blocks

_CACHE = {}

# evict engine per big chunk (cjp*NT+ni): s = scalar act (+vector bf16
# cb-add), v = vector fused scalar_tensor_tensor
_EV = "ssvssvssvssvssvs"
# sqm (mu^2) engine per c-chunk; sqm0 early (gpsimd), sqm1 on vector
# (scalar Square would thrash the activation table sets)
_SQM = ["g", "v", "g", "g"]


def _build():
    nc = bacc.Bacc("TRN2", target_bir_lowering=False, debug=False,
                   enable_asserts=False, num_devices=NCORES)

    # host-packed layouts (layout/dtype-only prep):
    #   mu8 [p, cj, kt, ci] fp8   (d = kt*128+p, c = cj*512+ci)
    #   zbf [p, kt, n]      bf16
    #   lcp [p, kt] f32, prp [p, ct] f32 (c = ct*128+p), prf [1, C] f32r
    mu8d = nc.dram_tensor("mu8", [P, CJ, KT, F], FP8, kind="ExternalInput").ap()
    zbfd = nc.dram_tensor("zbf", [P, KT, NSH], BF16, kind="ExternalInput").ap()
    lcp = nc.dram_tensor("lcp", [P, KT], F32, kind="ExternalInput").ap()
    prp = nc.dram_tensor("prp", [P, C // P], F32, kind="ExternalInput").ap()
    prf = nc.dram_tensor("prf", [1, C], F32R, kind="ExternalInput").ap()
    out = nc.dram_tensor("out", [NSH, C], BF16, kind="ExternalOutput").ap()

    with tile.TileContext(nc) as tc:
        with (
            tc.tile_pool(name="const", bufs=1) as const,
            tc.tile_pool(name="sq", bufs=2) as sq,
            tc.tile_pool(name="ob", bufs=2) as ob,
            tc.tile_pool(name="psM", bufs=6, space="PSUM") as psM,
            tc.tile_pool(name="psX", bufs=1, space="PSUM") as psX,
        ):
            # ---- input DMAs (parallel issue queues) --------------------
            mu8_s = const.tile([P, CJ, KT, F], FP8)
            zbf_s = const.tile([P, KT, NSH], BF16)
            nc.sync.dma_start(out=zbf_s[:, 0, :], in_=zbfd[:, 0, :])
            nc.gpsimd.dma_start(out=zbf_s[:, 1, :], in_=zbfd[:, 1, :])
            nc.sync.dma_start(out=mu8_s[:, 0], in_=mu8d[:, 0])
            nc.gpsimd.dma_start(out=zbf_s[:, 3, :], in_=zbfd[:, 3, :])
            nc.sync.dma_start(out=zbf_s[:, 2, :], in_=zbfd[:, 2, :])
            nc.gpsimd.dma_start(out=mu8_s[:, 1], in_=mu8d[:, 1])
            nc.sync.dma_start(out=mu8_s[:, 2], in_=mu8d[:, 2])
            nc.gpsimd.dma_start(out=mu8_s[:, 3], in_=mu8d[:, 3])

            lc_s = const.tile([P, KT], F32)
            nc.scalar.dma_start(out=lc_s[:], in_=lcp)
            prp_s = const.tile([P, C // P], F32)
            nc.scalar.dma_start(out=prp_s[:], in_=prp)
            prf_s = const.tile([1, C], F32R)
            nc.gpsimd.dma_start(out=prf_s[:], in_=prf)

            # ---- small constants ---------------------------------------
            onesb = const.tile([P, P], BF16)
            nc.vector.memset(onesb[:], 1.0)
            onesf = const.tile([1, P], F32)
            nc.vector.memset(onesf[:], 1.0)
            onescol = const.tile([P, 1], F32)
            nc.vector.memset(onescol[:], 1.0)
            id1 = const.tile([1, 1], F32)
            nc.vector.memset(id1[:], 1.0)

            # ---- scalar: iv chain --------------------------------------
            iv = const.tile([P, KT], F32)          # exp(-lc)
            nc.scalar.activation(iv[:], lc_s[:], AF.Exp, scale=-1.0)
            niv = const.tile([P, KT], F32)         # -0.5*iv
            nc.scalar.mul(niv[:], iv[:], -0.5)
            ivbf = const.tile([P, KT], BF16)
            nc.scalar.copy(ivbf[:], iv[:])
            ones1r = const.tile([1, P], F32R)
            nc.scalar.copy(ones1r[:], onesf[:])

            # lse/log_det seeds: exp+row-reduce fused on scalar, log_det
            # row-reduce on vector (both tiny, no gpsimd involved).
            # prior logits are O(1) so exp without max-subtraction is safe.
            CT = C // P
            e128 = const.tile([P, CT], F32)
            esldp = const.tile([P, 8], F32)
            nc.vector.memset(esldp[:], 0.0)
            nc.scalar.activation(e128[:], prp_s[:], AF.Exp, scale=1.0,
                                 accum_out=esldp[:, 0:1])
            nc.vector.tensor_reduce(out=esldp[:, 1:2], in_=lc_s[:],
                                    axis=mybir.AxisListType.X, op=ALU.add)

            # ---- z prep (vector): zs8 feeds the mains ------------------
            zs8 = const.tile([P, KT, NSH], FP8)
            for kt in range(KT):
                nc.vector.tensor_scalar_mul(zs8[:, kt, :], zbf_s[:, kt, :],
                                            iv[:, kt:kt + 1])

            # sqm tiles; gpsimd tensor ops poison SBUF for DVE/scalar
            # (5x slowdowns observed) so they live on vector + scalar
            sqm_t = [None] * CJ
            for cj in range(CJ):
                sqm_t[cj] = sq.tile([P, KT, F], FP8, tag=f"sqm{cj % 2}",
                                    name=f"sqm{cj}")

            cb_bf = const.tile([P, C], BF16)
            rb = const.tile([P, NT], F32)
            zq = const.tile([P, KT, NSH], BF16)
            zsqf = const.tile([1, NSH], F32)
            betab = const.tile([P, 1], F32)

            def emit_main(cjp, ni, obf):
                pss = []
                nsl = slice(ni * P, (ni + 1) * P)
                for half in range(2):
                    ps = psM.tile([P, F], F32)
                    cj = 2 * cjp + half
                    for j in range(2):
                        nc.tensor.matmul(
                            ps[:],
                            lhsT=zs8[:, 2 * j:2 * j + 2, nsl],
                            rhs=mu8_s[:, cj, 2 * j:2 * j + 2, :],
                            start=(j == 0), stop=(j == 1),
                            perf_mode=DR)
                    pss.append(ps)
                return pss

            def emit_zq_half(h):
                s = slice(h * (NSH // 2), (h + 1) * (NSH // 2))
                nc.vector.tensor_tensor(zq[:, :, s], zbf_s[:, :, s],
                                        zbf_s[:, :, s], ALU.mult)
                pz = psX.tile([P, NSH // 2], F32, tag="x",
                              name=f"pz{h}")
                for kt in range(KT):
                    nc.tensor.matmul(pz[0:1, :], lhsT=ivbf[:, kt:kt + 1],
                                     rhs=zq[:, kt, s],
                                     start=(kt == 0), stop=(kt == KT - 1))
                nc.scalar.activation(zsqf[:, s], pz[0:1, :], AF.Identity,
                                     scale=-0.5)
                for i in range(NT // 2):
                    ni = h * (NT // 2) + i
                    nc.tensor.transpose(pz[:, 400 + i:401 + i],
                                        zsqf[:, ni * P:(ni + 1) * P], id1[:])
                nc.scalar.activation(rb[:, h * 4:h * 4 + 4],
                                     pz[:, 400:404], AF.Identity,
                                     bias=betab[:], scale=1.0)

            def emit_psc(cj):
                pc = psX.tile([P, F], F32, tag="x", name=f"pc{cj}")
                for j in range(2):
                    nc.tensor.matmul(pc[:], lhsT=W[:, 2 * j:2 * j + 2, :],
                                     rhs=sqm_t[cj][:, 2 * j:2 * j + 2, :],
                                     start=(j == 0), stop=False,
                                     perf_mode=DR)
                nc.tensor.matmul(pc[:], lhsT=ones1r[:],
                                 rhs=prf_s[:, cj * F:(cj + 1) * F],
                                 start=False, stop=True)
                nc.scalar.activation(cb_bf[:, cj * F:(cj + 1) * F],
                                     pc[:], AF.Identity, scale=1.0)

            def emit_evict(cjp, ni, pss, obf):
                for half in range(2):
                    cj = 2 * cjp + half
                    hsl = slice(half * F, (half + 1) * F)
                    cbs = cb_bf[:, cj * F:(cj + 1) * F]
                    ob_h = obf[:, ni, hsl]
                    if _EV[(2 * (cjp * NT + ni) + half) % 16] == "s":
                        nc.scalar.activation(ob_h, pss[half][:], AF.Identity,
                                             bias=rb[:, ni:ni + 1], scale=1.0)
                        nc.vector.tensor_tensor(ob_h, ob_h, cbs, ALU.add)
                    else:
                        nc.vector.scalar_tensor_tensor(ob_h, pss[half][:],
                                                       rb[:, ni:ni + 1],
                                                       cbs,
                                                       op0=ALU.add,
                                                       op1=ALU.add)

            def emit_store(cjp, ni, obf):
                nc.sync.dma_start(
                    out=out[ni * P:(ni + 1) * P, cjp * F2:(cjp + 1) * F2],
                    in_=obf[:, ni])

            # ---- betab first: PE is idle before the mains anyway ------
            bm = psX.tile([P, F], F32, tag="x", name="bm")
            nc.tensor.matmul(bm[0:1, 0:8], lhsT=onescol[:], rhs=esldp[:],
                             start=True, stop=True)
            lse1 = const.tile([1, 1], F32)
            nc.scalar.activation(lse1[:], bm[0:1, 0:1], AF.Ln)
            zros8 = const.tile([1, 8], F32)
            nc.vector.memset(zros8[:], 0.0)
            bun = const.tile([1, 8], F32R)
            nc.scalar.copy(bun[:], zros8[:])
            # bun[0] = -(0.5*log_det + lse)
            nc.scalar.activation(bun[:, 0:1], bm[0:1, 1:2], AF.Identity,
                                 bias=lse1[:], scale=0.5)
            nc.scalar.activation(bun[:, 0:1], bun[:, 0:1], AF.Identity,
                                 scale=-1.0)
            nc.tensor.matmul(bm[:, 8:16], lhsT=ones1r[:], rhs=bun[:],
                             start=True, stop=True)
            nc.scalar.copy(betab[:], bm[:, 8:9])

            # scalar: sqm0 square early (set2 table is now loaded)
            nc.scalar.activation(sqm_t[0][:], mu8_s[:, 0], AF.Square)

            # DVE stream: zq-h0, W, sqm1, zq-h1 (zs8 emitted above)
            obf0 = ob.tile([P, NT, F2], BF16, tag="ob0")
            ps_q = [emit_main(0, ni, obf0) for ni in range(3)]
            emit_zq_half(0)
            W = const.tile([P, KT, P], FP8)
            for kt in range(KT):
                nc.vector.tensor_scalar_mul(W[:, kt, :], onesb[:],
                                            niv[:, kt:kt + 1])
            emit_psc(0)
            nc.vector.tensor_tensor(sqm_t[1][:], mu8_s[:, 1], mu8_s[:, 1],
                                    ALU.mult)
            ps_q.append(emit_main(0, 3, obf0))
            emit_psc(1)
            emit_evict(0, 0, ps_q[0], obf0)
            emit_store(0, 0, obf0)
            emit_zq_half(1)
            emit_evict(0, 1, ps_q[1], obf0)
            emit_store(0, 1, obf0)
            nc.scalar.activation(sqm_t[2][:], mu8_s[:, 2], AF.Square)
            for ni in range(2, 4):
                emit_evict(0, ni, ps_q[ni], obf0)
                emit_store(0, ni, obf0)
            nc.vector.tensor_tensor(sqm_t[3][:], mu8_s[:, 3], mu8_s[:, 3],
                                    ALU.mult)
            for ni in range(4, NT):
                ps = emit_main(0, ni, obf0)
                if ni == 5:
                    emit_psc(2)
                emit_evict(0, ni, ps, obf0)
                emit_store(0, ni, obf0)
            obf1 = ob.tile([P, NT, F2], BF16, tag="ob1")
            for ni in range(NT):
                ps = emit_main(1, ni, obf1)
                if ni == 0:
                    emit_psc(3)
                emit_evict(1, ni, ps, obf1)
                emit_store(1, ni, obf1)

    nc.compile()
    return nc


def _get_nc():
    if "nc" not in _CACHE:
        _CACHE["nc"] = _build()
    return _CACHE["nc"]


def _round_f32r(a):
    u = np.ascontiguousarray(a, dtype=np.float32).view(np.uint32)
    r = (u + np.uint32(0x1FF) + ((u >> np.uint32(10)) & np.uint32(1))) \
        & np.uint32(0xFFFFFC00)
    return r.view(np.float32)


def _in_maps(z, mu, log_cov_diag, prior_logits):
    z = np.asarray(z, dtype=np.float32)
    mu = np.asarray(mu, dtype=np.float32)
    lc = np.asarray(log_cov_diag, dtype=np.float32)
    pl = np.asarray(prior_logits, dtype=np.float32)

    mu8 = np.ascontiguousarray(
        mu.T.reshape(KT, P, CJ, F).transpose(1, 2, 0, 3)
    ).astype(ml_dtypes.float8_e4m3)
    lcp = np.ascontiguousarray(lc.reshape(KT, P).T)
    prp = np.ascontiguousarray(pl.reshape(C // P, P).T)
    prf = _round_f32r(pl.reshape(1, C))
    maps = []
    for c in range(NCORES):
        zc = z[c * NSH:(c + 1) * NSH, :]
        zbf = np.ascontiguousarray(
            zc.T.reshape(KT, P, NSH).transpose(1, 0, 2)
        ).astype(ml_dtypes.bfloat16)
        maps.append({"mu8": mu8, "zbf": zbf, "lcp": lcp, "prp": prp,
                     "prf": prf})
    return maps


def _run(z, mu, log_cov_diag, prior_logits, trace=False, **kw):
    nc = _get_nc()
    maps = _in_maps(z, mu, log_cov_diag, prior_logits)
    res = run_bass_kernel_spmd(nc, maps, list(range(NCORES)), trace=trace, **kw)
    full = np.concatenate(
        [res.results[c]["out"].astype(np.float32) for c in range(NCORES)],
        axis=0)
    return full, res


def kernel(z, mu, log_cov_diag, prior_logits):
    full, _ = _run(z, mu, log_cov_diag, prior_logits)
    return full


# revision 37
# speedup vs baseline: 1.1738x; 1.1738x over previous
"""Trainium2 Bass kernel for DiagTrainableLDAHead (retrieval_knn).

out[n,c] = log_prior[c] - 0.5*(m2[n,c] + log_det)
m2[n,c]  = sum_d (z[n,d]-mu[c,d])^2 * inv_var[d]

=> out[n,c] = cross[n,c] + cb[c] + rb[n]
   cross = (z*iv) @ mu^T          fp8 DoubleRow GEMM (2 k-passes of 256)
   cb[c] = prior[c] - 0.5*sum_d mu[c,d]^2 iv[d]
           computed on the PE ([128,2,1] -iv/2 stationary over mu^2 fp8 +
           prior rank-1), then ACCUMULATED INTO EACH PSUM CHUNK by a
           trailing fp32r rank-1 matmul (ones x cb) in the same
           accumulation group -- so eviction needs no second tensor input.
   rb[n] = -0.5*sum_d z[n,d]^2 iv[d] - logsumexp(prior) - 0.5*log_det
           applied as the [P,1] bias of the single-pass eviction.

Eviction: psum [128,1024] (2 banks) -> bf16 SBUF, alternating
scalar.activation(bias=rb) / vector.tensor_scalar_add(rb). gpsimd cannot
read PSUM (hardware restriction) so it handles SBUF-only mu^2 squares.

Sharding: data-parallel over N across 8 cores; mu/log_cov/prior
replicated; no collectives. Host prep is layout/dtype-only; output is
computed in bf16 and widened to fp32 on the host (|out|~350, bf16 ulp~1,
tolerance ~7).
"""
import sys

sys.path.insert(0, "/opt/trn_rl_repo")

import numpy as np
import ml_dtypes

import concourse.bacc as bacc
import concourse.tile as tile
from concourse import mybir
from concourse import bass_isa
from concourse.bass_utils import run_bass_kernel_spmd

F32 = mybir.dt.float32
F32R = mybir.dt.float32r
BF16 = mybir.dt.bfloat16
FP8 = mybir.dt.float8e4
AF = mybir.ActivationFunctionType
ALU = mybir.AluOpType
DR = mybir.MatmulPerfMode.DoubleRow

N, C, D = 8192, 2048, 512
NCORES = 8
NSH = N // NCORES          # 1024 rows per core
P = 128                    # partitions
KT = D // P                # 4 k-tiles
NT = NSH // P              # 8 n-tiles
F = 512                    # single PSUM bank chunk
F2 = 2 * F                 # eviction chunk (2 banks)
CJ = C // F                # 4 c-chunks
CP = C // F2               # 2 c-pair blocks

_CACHE = {}

# evict engine per big chunk (cjp*NT+ni): s = scalar act (+vector bf16
# cb-add), v = vector fused scalar_tensor_tensor
_EV = "ssvssvssvssvssvs"
# sqm (mu^2) engine per c-chunk; sqm0 early (gpsimd), sqm1 on vector
# (scalar Square would thrash the activation table sets)
_SQM = ["g", "v", "g", "g"]


def _build():
    nc = bacc.Bacc("TRN2", target_bir_lowering=False, debug=False,
                   enable_asserts=False, num_devices=NCORES)

    # host-packed layouts (layout/dtype-only prep):
    #   mu8 [p, cj, kt, ci] fp8   (d = kt*128+p, c = cj*512+ci)
    #   zbf [p, kt, n]      bf16
    #   lcp [p, kt] f32, prp [p, ct] f32 (c = ct*128+p), prf [1, C] f32r
    mu8d = nc.dram_tensor("mu8", [P, CJ, KT, F], FP8, kind="ExternalInput").ap()
    zbfd = nc.dram_tensor("zbf", [P, KT, NSH], BF16, kind="ExternalInput").ap()
    lcp = nc.dram_tensor("lcp", [P, KT], F32, kind="ExternalInput").ap()
    prp = nc.dram_tensor("prp", [P, C // P], F32, kind="ExternalInput").ap()
    prf = nc.dram_tensor("prf", [1, C], F32R, kind="ExternalInput").ap()
    out = nc.dram_tensor("out", [NSH, C], BF16, kind="ExternalOutput").ap()

    with tile.TileContext(nc) as tc:
        with (
            tc.tile_pool(name="const", bufs=1) as const,
            tc.tile_pool(name="sq", bufs=2) as sq,
            tc.tile_pool(name="ob", bufs=2) as ob,
            tc.tile_pool(name="psM", bufs=3, space="PSUM") as psM,
            tc.tile_pool(name="psC", bufs=1, space="PSUM") as psC,
            tc.tile_pool(name="psX", bufs=1, space="PSUM") as psX,
        ):
            # ---- input DMAs (parallel issue queues) --------------------
            mu8_s = const.tile([P, CJ, KT, F], FP8)
            zbf_s = const.tile([P, KT, NSH], BF16)
            nc.sync.dma_start(out=zbf_s[:, 0, :], in_=zbfd[:, 0, :])
            nc.gpsimd.dma_start(out=zbf_s[:, 1, :], in_=zbfd[:, 1, :])
            nc.sync.dma_start(out=mu8_s[:, 0], in_=mu8d[:, 0])
            nc.gpsimd.dma_start(out=zbf_s[:, 3, :], in_=zbfd[:, 3, :])
            nc.sync.dma_start(out=zbf_s[:, 2, :], in_=zbfd[:, 2, :])
            nc.gpsimd.dma_start(out=mu8_s[:, 1], in_=mu8d[:, 1])
            nc.sync.dma_start(out=mu8_s[:, 2], in_=mu8d[:, 2])
            nc.gpsimd.dma_start(out=mu8_s[:, 3], in_=mu8d[:, 3])

            lc_s = const.tile([P, KT], F32)
            nc.scalar.dma_start(out=lc_s[:], in_=lcp)
            prp_s = const.tile([P, C // P], F32)
            nc.scalar.dma_start(out=prp_s[:], in_=prp)
            prf_s = const.tile([1, C], F32R)
            nc.gpsimd.dma_start(out=prf_s[:], in_=prf)

            # ---- small constants ---------------------------------------
            onesb = const.tile([P, P], BF16)
            nc.vector.memset(onesb[:], 1.0)
            onesf = const.tile([1, P], F32)
            nc.vector.memset(onesf[:], 1.0)
            onescol = const.tile([P, 1], F32)
            nc.vector.memset(onescol[:], 1.0)
            id1 = const.tile([1, 1], F32)
            nc.vector.memset(id1[:], 1.0)

            # ---- scalar: iv chain --------------------------------------
            iv = const.tile([P, KT], F32)          # exp(-lc)
            nc.scalar.activation(iv[:], lc_s[:], AF.Exp, scale=-1.0)
            niv = const.tile([P, KT], F32)         # -0.5*iv
            nc.scalar.mul(niv[:], iv[:], -0.5)
            ivbf = const.tile([P, KT], BF16)
            nc.scalar.copy(ivbf[:], iv[:])
            ones1r = const.tile([1, P], F32R)
            nc.scalar.copy(ones1r[:], onesf[:])

            # lse/log_det seeds: exp+row-reduce fused on scalar, log_det
            # row-reduce on vector (both tiny, no gpsimd involved).
            # prior logits are O(1) so exp without max-subtraction is safe.
            CT = C // P
            e128 = const.tile([P, CT], F32)
            esldp = const.tile([P, 8], F32)
            nc.vector.memset(esldp[:], 0.0)
            nc.scalar.activation(e128[:], prp_s[:], AF.Exp, scale=1.0,
                                 accum_out=esldp[:, 0:1])
            nc.vector.tensor_reduce(out=esldp[:, 1:2], in_=lc_s[:],
                                    axis=mybir.AxisListType.X, op=ALU.add)

            # ---- z prep (vector): zs8 feeds the mains ------------------
            zs8 = const.tile([P, KT, NSH], FP8)
            for kt in range(KT):
                nc.vector.tensor_scalar_mul(zs8[:, kt, :], zbf_s[:, kt, :],
                                            iv[:, kt:kt + 1])

            # sqm tiles; gpsimd tensor ops poison SBUF for DVE/scalar
            # (5x slowdowns observed) so they live on vector + scalar
            sqm_t = [None] * CJ
            for cj in range(CJ):
                sqm_t[cj] = sq.tile([P, KT, F], FP8, tag=f"sqm{cj % 2}",
                                    name=f"sqm{cj}")

            cb_bf = const.tile([P, C], BF16)
            rb = const.tile([P, NT], F32)
            zq = const.tile([P, KT, NSH], BF16)
            zsqf = const.tile([1, NSH], F32)
            betab = const.tile([P, 1], F32)

            def emit_main(cjp, ni, obf):
                ps = psM.tile([P, F2], F32)
                nsl = slice(ni * P, (ni + 1) * P)
                for half in range(2):
                    cj = 2 * cjp + half
                    hsl = slice(half * F, (half + 1) * F)
                    for j in range(2):
                        nc.tensor.matmul(
                            ps[:, hsl],
                            lhsT=zs8[:, 2 * j:2 * j + 2, nsl],
                            rhs=mu8_s[:, cj, 2 * j:2 * j + 2, :],
                            start=(j == 0), stop=(j == 1),
                            perf_mode=DR)
                return ps

            def emit_zq_half(h):
                s = slice(h * (NSH // 2), (h + 1) * (NSH // 2))
                nc.vector.tensor_tensor(zq[:, :, s], zbf_s[:, :, s],
                                        zbf_s[:, :, s], ALU.mult)
                pz = psX.tile([P, NSH // 2], F32, tag="x",
                              name=f"pz{h}")
                for kt in range(KT):
                    nc.tensor.matmul(pz[0:1, :], lhsT=ivbf[:, kt:kt + 1],
                                     rhs=zq[:, kt, s],
                                     start=(kt == 0), stop=(kt == KT - 1))
                nc.scalar.activation(zsqf[:, s], pz[0:1, :], AF.Identity,
                                     scale=-0.5)
                for i in range(NT // 2):
                    ni = h * (NT // 2) + i
                    nc.tensor.transpose(pz[:, 400 + i:401 + i],
                                        zsqf[:, ni * P:(ni + 1) * P], id1[:])
                nc.scalar.activation(rb[:, h * 4:h * 4 + 4],
                                     pz[:, 400:404], AF.Identity,
                                     bias=betab[:], scale=1.0)

            def emit_psc(cj):
                pc = psC.tile([P, F], F32, tag="cb")
                for j in range(2):
                    nc.tensor.matmul(pc[:], lhsT=W[:, 2 * j:2 * j + 2, :],
                                     rhs=sqm_t[cj][:, 2 * j:2 * j + 2, :],
                                     start=(j == 0), stop=False,
                                     perf_mode=DR)
                nc.tensor.matmul(pc[:], lhsT=ones1r[:],
                                 rhs=prf_s[:, cj * F:(cj + 1) * F],
                                 start=False, stop=True)
                nc.scalar.activation(cb_bf[:, cj * F:(cj + 1) * F],
                                     pc[:], AF.Identity, scale=1.0)

            def emit_evict(cjp, ni, ps, obf):
                cbs = cb_bf[:, cjp * F2:(cjp + 1) * F2]
                if _EV[(cjp * NT + ni) % 16] == "s":
                    nc.scalar.activation(obf[:, ni], ps[:], AF.Identity,
                                         bias=rb[:, ni:ni + 1], scale=1.0)
                    nc.vector.tensor_tensor(obf[:, ni], obf[:, ni],
                                            cbs, ALU.add)
                else:
                    nc.vector.scalar_tensor_tensor(obf[:, ni], ps[:],
                                                   rb[:, ni:ni + 1],
                                                   cbs,
                                                   op0=ALU.add, op1=ALU.add)

            def emit_store(cjp, ni, obf):
                nc.sync.dma_start(
                    out=out[ni * P:(ni + 1) * P, cjp * F2:(cjp + 1) * F2],
                    in_=obf[:, ni])

            # ---- betab first: PE is idle before the mains anyway ------
            bm = psX.tile([P, F], F32, tag="x", name="bm")
            nc.tensor.matmul(bm[0:1, 0:8], lhsT=onescol[:], rhs=esldp[:],
                             start=True, stop=True)
            lse1 = const.tile([1, 1], F32)
            nc.scalar.activation(lse1[:], bm[0:1, 0:1], AF.Ln)
            zros8 = const.tile([1, 8], F32)
            nc.vector.memset(zros8[:], 0.0)
            bun = const.tile([1, 8], F32R)
            nc.scalar.copy(bun[:], zros8[:])
            # bun[0] = -(0.5*log_det + lse)
            nc.scalar.activation(bun[:, 0:1], bm[0:1, 1:2], AF.Identity,
                                 bias=lse1[:], scale=0.5)
            nc.scalar.activation(bun[:, 0:1], bun[:, 0:1], AF.Identity,
                                 scale=-1.0)
            nc.tensor.matmul(bm[:, 8:16], lhsT=ones1r[:], rhs=bun[:],
                             start=True, stop=True)
            nc.scalar.copy(betab[:], bm[:, 8:9])

            # scalar: sqm0 square early (set2 table is now loaded)
            nc.scalar.activation(sqm_t[0][:], mu8_s[:, 0], AF.Square)

            # DVE stream: zq-h0, W, sqm1, zq-h1 (zs8 emitted above)
            obf0 = ob.tile([P, NT, F2], BF16, tag="ob0")
            ps_q = [emit_main(0, ni, obf0) for ni in range(3)]
            emit_zq_half(0)
            W = const.tile([P, KT, P], FP8)
            for kt in range(KT):
                nc.vector.tensor_scalar_mul(W[:, kt, :], onesb[:],
                                            niv[:, kt:kt + 1])
            emit_psc(0)
            nc.vector.tensor_tensor(sqm_t[1][:], mu8_s[:, 1], mu8_s[:, 1],
                                    ALU.mult)
            ps_q.append(emit_main(0, 3, obf0))
            emit_psc(1)
            emit_evict(0, 0, ps_q[0], obf0)
            emit_store(0, 0, obf0)
            emit_zq_half(1)
            emit_evict(0, 1, ps_q[1], obf0)
            emit_store(0, 1, obf0)
            nc.scalar.activation(sqm_t[2][:], mu8_s[:, 2], AF.Square)
            for ni in range(2, 4):
                emit_evict(0, ni, ps_q[ni], obf0)
                emit_store(0, ni, obf0)
            nc.vector.tensor_tensor(sqm_t[3][:], mu8_s[:, 3], mu8_s[:, 3],
                                    ALU.mult)
            for ni in range(4, NT):
                ps = emit_main(0, ni, obf0)
                if ni == 5:
                    emit_psc(2)
                emit_evict(0, ni, ps, obf0)
                emit_store(0, ni, obf0)
            obf1 = ob.tile([P, NT, F2], BF16, tag="ob1")
            for ni in range(NT):
                ps = emit_main(1, ni, obf1)
                if ni == 0:
                    emit_psc(3)
                emit_evict(1, ni, ps, obf1)
                emit_store(1, ni, obf1)

    nc.compile()
    return nc


def _get_nc():
    if "nc" not in _CACHE:
        _CACHE["nc"] = _build()
    return _CACHE["nc"]


def _round_f32r(a):
    u = np.ascontiguousarray(a, dtype=np.float32).view(np.uint32)
    r = (u + np.uint32(0x1FF) + ((u >> np.uint32(10)) & np.uint32(1))) \
        & np.uint32(0xFFFFFC00)
    return r.view(np.float32)


def _in_maps(z, mu, log_cov_diag, prior_logits):
    z = np.asarray(z, dtype=np.float32)
    mu = np.asarray(mu, dtype=np.float32)
    lc = np.asarray(log_cov_diag, dtype=np.float32)
    pl = np.asarray(prior_logits, dtype=np.float32)

    mu8 = np.ascontiguousarray(
        mu.T.reshape(KT, P, CJ, F).transpose(1, 2, 0, 3)
    ).astype(ml_dtypes.float8_e4m3)
    lcp = np.ascontiguousarray(lc.reshape(KT, P).T)
    prp = np.ascontiguousarray(pl.reshape(C // P, P).T)
    prf = _round_f32r(pl.reshape(1, C))
    maps = []
    for c in range(NCORES):
        zc = z[c * NSH:(c + 1) * NSH, :]
        zbf = np.ascontiguousarray(
            zc.T.reshape(KT, P, NSH).transpose(1, 0, 2)
        ).astype(ml_dtypes.bfloat16)
        maps.append({"mu8": mu8, "zbf": zbf, "lcp": lcp, "prp": prp,
                     "prf": prf})
    return maps


def _run(z, mu, log_cov_diag, prior_logits, trace=False, **kw):
    nc = _get_nc()
    maps = _in_maps(z, mu, log_cov_diag, prior_logits)
    res = run_bass_kernel_spmd(nc, maps, list(range(NCORES)), trace=trace, **kw)
    full = np.concatenate(
        [res.results[c]["out"].astype(np.float32) for c in range(NCORES)],
        axis=0)
    return full, res


def kernel(z, mu, log_cov_diag, prior_logits):
    full, _ = _run(z, mu, log_cov_diag, prior_logits)
    return full
